# revision 11
# baseline (speedup 1.0000x reference)
"""GNN NodeBlock kernel for Trainium2, 8 NeuronCores (SPMD, no collectives).

Reference computation (N=50000 nodes, E=1600000 edges, F=128 features):
    recv_agg = segment_sum(edge_attr, edge_index[1], N)        # [N, 128]
    collected = concat([recv_agg, x, broadcast(u)], -1)        # [N, 272]
    out = relu(collected @ W1 + b1) @ W2 + b2                  # [N, 128]

Host-side sharding: nodes are re-balanced into 1584 bins of <=32 nodes
with <=1024 edges each (snake assignment of degree-sorted nodes), so
every bin needs exactly 8 edge-chunks of 128 — a fully static, uniform
program across all 8 cores (198 bins/core, 1584 chunks/core vs ~1758
for contiguous sharding).  Edges are packed fp8(e4m3) with
error-feedback quantization per receiver run: the device's fp32 PSUM
sum of the quantized stream telescopes to the true sum minus one final
sub-ulp carry (rel err ~6e-3 vs 2.9e-2 for plain fp8 rounding).  The
u-term of layer 1 is folded into b1 on the host.

Device program per core, one supertile (16 bins = 512 nodes) at a time:
  scatter: one DVE is_equal builds all 128 one-hot blocks
    onehot[e, q, n] = (rel[e, q] == n) in fp8; the PE accumulates
    aggT[f, 32-node window] += chunk^T @ onehot into a [128, 512] PSUM
    bank (8 chunks per bin, start/stop per bin).  fp8 weights get FWL
    (4 elem/cycle LDWEIGHTS), so the 32-wide matmuls run at the ~60
    cycle dispatch floor.
  L1: h_ps[h, n] = W1a^T @ aggT + W1b^T @ xT per 128-hidden chunk;
    ReLU+bias evacuation alternates scalar/vector engines.
  L2: o_ps[f, n] = sum_hc W2r_hc^T @ hT_hc (weights stationary,
    feature-major output); bias b2 added during PSUM evacuation; host
    transposes the output back to node-major.
Everything streams behind the PE: the edge DMA (2 MB fp8/supertile) and
all DVE/ACT work fit well under the ~8.4 us/supertile of matmul.
"""

import numpy as np

from concourse import bacc, mybir, tile
from concourse import bass_utils
from concourse.bass_interp import get_hw_module

# ---------------- problem constants (hardcoded per spec) ----------------
N_NODES = 50000
N_EDGES = 1600000
F = 128           # edge/node feature dim
H = 1024          # hidden dim
HC = H // 128     # 8 hidden chunks
D_U = 16
N_CORES = 8
TN = 32                                # nodes per bin (one-hot window)
NT = 198                               # bins per core
B_GLOBAL = N_CORES * NT                # 1584 bins
CPB = 8                                # chunks per bin (bin cap = 1024 edges)
QT = NT * CPB                          # 1584 chunks per core
NODES_PAD = NT * TN                    # 6336 node slots per core
SUP = 16                               # bins per supertile (512 nodes)
# graded startup: the NEFF's DMA path has ~9us of fixed init latency and
# then ramps; small first supertiles let the PE start on the first
# 0.25 MB slab instead of stalling on a 2 MB one
SUPERS = [2, 2, 4, 8, 12] + [SUP] * 10 + [NT - 28 - SUP * 10]
assert sum(SUPERS) == NT

EDGE_DT = mybir.dt.float8e4            # ml_dtypes.float8_e4m3
IDX_DT = mybir.dt.bfloat16             # rel codes / iota (ints 0..31 exact)
MLP_DT = mybir.dt.bfloat16
OUT_DT = mybir.dt.bfloat16             # on-device output store dtype

_np = mybir.dt.np  # mybir dtype -> numpy dtype


# ---------------- device program ----------------

def build_program():
    f32 = mybir.dt.float32

    nc = bacc.Bacc("TRN2", target_bir_lowering=False, debug=False,
                   num_devices=N_CORES)

    edges = nc.dram_tensor("edges", [128, QT, F], EDGE_DT,
                           kind="ExternalInput").ap()
    relT = nc.dram_tensor("relT", [128, QT], IDX_DT,
                          kind="ExternalInput").ap()
    iota = nc.dram_tensor("iota", [128, TN], IDX_DT,
                          kind="ExternalInput").ap()
    xT = nc.dram_tensor("xT", [128, NODES_PAD], MLP_DT,
                        kind="ExternalInput").ap()
    w1a = nc.dram_tensor("w1a", [128, H], MLP_DT, kind="ExternalInput").ap()
    w1b = nc.dram_tensor("w1b", [128, H], MLP_DT, kind="ExternalInput").ap()
    w2r = nc.dram_tensor("w2r", [128, H], MLP_DT, kind="ExternalInput").ap()
    b1T = nc.dram_tensor("b1T", [128, HC], f32, kind="ExternalInput").ap()
    b2T = nc.dram_tensor("b2T", [128, 1], f32, kind="ExternalInput").ap()
    y = nc.dram_tensor("y", [128, NODES_PAD], OUT_DT,
                       kind="ExternalOutput").ap()

    with tile.TileContext(nc) as tc:
        with (
            tc.tile_pool(name="const", bufs=1) as cpool,
            tc.tile_pool(name="edge", bufs=4) as epool,
            tc.tile_pool(name="oh", bufs=3) as ohpool,
            tc.tile_pool(name="agg", bufs=2) as aggpool,
            tc.tile_pool(name="h", bufs=2) as hpool,
            tc.tile_pool(name="out", bufs=2) as outpool,
            tc.tile_pool(name="ps_agg", bufs=2, space="PSUM") as ps_agg,
            tc.tile_pool(name="ps_h", bufs=4, space="PSUM") as ps_h,
            tc.tile_pool(name="ps_out", bufs=2, space="PSUM") as ps_out,
        ):
            # rel codes + iota lead the sync ring (the one-hot for the
            # first supertile needs them); edge slabs follow.  MLP
            # weights stream on the scalar ring concurrently, x head
            # first so L1 of the small first supertiles isn't gated on
            # the full 1.6 MB xT load.
            relT_sb = cpool.tile([128, QT], IDX_DT, tag="relT")
            nc.sync.dma_start(relT_sb[:], relT[:])
            iota_sb = cpool.tile([128, TN], IDX_DT, tag="iota")
            nc.sync.dma_start(iota_sb[:], iota[:])
            w1a_sb = cpool.tile([128, H], MLP_DT, tag="w1a")
            nc.scalar.dma_start(w1a_sb[:], w1a[:])
            w1b_sb = cpool.tile([128, H], MLP_DT, tag="w1b")
            nc.scalar.dma_start(w1b_sb[:], w1b[:])
            XH = 512  # covers the first two supertiles (16 bins)
            xT_sb = cpool.tile([128, NODES_PAD], MLP_DT, tag="xT")
            nc.scalar.dma_start(xT_sb[:, :XH], xT[:, :XH])
            b1T_sb = cpool.tile([128, HC], f32, tag="b1T")
            nc.scalar.dma_start(b1T_sb[:], b1T[:])
            nc.scalar.dma_start(xT_sb[:, XH:], xT[:, XH:])
            w2r_sb = cpool.tile([128, H], MLP_DT, tag="w2r")
            nc.scalar.dma_start(w2r_sb[:], w2r[:])
            b2T_sb = cpool.tile([128, 1], f32, tag="b2T")
            nc.scalar.dma_start(b2T_sb[:], b2T[:])

            iota_bc1 = iota_sb[:].rearrange("p (u n) -> p u n", u=1)

            # PE warm-up: engines start ~3us into the NEFF but the first
            # DMA bytes only land ~9us in.  Junk matmuls on a memset
            # scratch (no DMA dependency) keep HAM at 8/8 through that
            # window so the first real supertiles don't run at half
            # clock.
            scratch = cpool.tile([128, 512], MLP_DT, tag="scratch")
            nc.vector.memset(scratch[:], 0.0)
            warm_ps = ps_agg.tile([128, 512], f32, tag="agg")
            for w in range(22):
                nc.tensor.matmul(warm_ps[:],
                                 lhsT=scratch[:, :128], rhs=scratch[:],
                                 start=True, stop=True)

            def make_onehot(s):
                nts_ = SUPERS[s]
                nq_ = nts_ * CPB
                q0_ = sum(SUPERS[:s]) * CPB
                oh_ = ohpool.tile([128, nq_, TN], EDGE_DT, tag="oh")
                rel_bc = relT_sb[:, q0_:q0_ + nq_].rearrange(
                    "p (c u) -> p c u", u=1).broadcast_to([128, nq_, TN])
                nc.vector.tensor_tensor(
                    out=oh_[:], in0=iota_bc1.broadcast_to([128, nq_, TN]),
                    in1=rel_bc, op=mybir.AluOpType.is_equal)
                return oh_

            ohs = {0: make_onehot(0)}
            t0 = 0
            for s, nts in enumerate(SUPERS):
                nn = nts * TN
                nq = nts * CPB
                q0 = t0 * CPB
                n0 = t0 * TN
                e_sup = epool.tile([128, nq, F], EDGE_DT, tag="e")
                nc.sync.dma_start(e_sup[:], edges[:, q0:q0 + nq])
                # build next supertile's one-hots on DVE while the PE
                # scatters this one
                oh = ohs.pop(s)
                if s + 1 < len(SUPERS):
                    ohs[s + 1] = make_onehot(s + 1)
                # scatter-sum into one PSUM bank, 32-col window per bin
                agg_ps = ps_agg.tile([128, nn], f32, tag="agg")
                for st in range(nts):
                    for c in range(CPB):
                        q = st * CPB + c
                        nc.tensor.matmul(
                            agg_ps[:, st * TN:(st + 1) * TN],
                            lhsT=e_sup[:, q, :],
                            rhs=oh[:, q, :],
                            start=(c == 0),
                            stop=(c == CPB - 1),
                        )
                aggT = aggpool.tile([128, nn], MLP_DT, tag="aggT")
                nc.scalar.copy(aggT[:], agg_ps[:])
                # layer 1, hidden chunk by hidden chunk
                hT = hpool.tile([128, HC, nn], MLP_DT, tag="hT")
                for hc in range(HC):
                    h_ps = ps_h.tile([128, nn], f32, tag="h")
                    nc.tensor.matmul(h_ps[:],
                                     lhsT=w1a_sb[:, hc * 128:(hc + 1) * 128],
                                     rhs=aggT[:],
                                     start=True, stop=False)
                    nc.tensor.matmul(h_ps[:],
                                     lhsT=w1b_sb[:, hc * 128:(hc + 1) * 128],
                                     rhs=xT_sb[:, n0:n0 + nn],
                                     start=False, stop=True)
                    if hc % 2 == 0:
                        nc.scalar.activation(
                            hT[:, hc, :], h_ps[:],
                            mybir.ActivationFunctionType.Relu,
                            bias=b1T_sb[:, hc:hc + 1], scale=1.0)
                    else:
                        nc.vector.tensor_scalar(
                            out=hT[:, hc, :], in0=h_ps[:],
                            scalar1=b1T_sb[:, hc:hc + 1], scalar2=0.0,
                            op0=mybir.AluOpType.add,
                            op1=mybir.AluOpType.max)
                # layer 2: weights stationary, feature-major output
                o_ps = ps_out.tile([128, nn], f32, tag="ops")
                for hc in range(HC):
                    nc.tensor.matmul(
                        o_ps[:],
                        lhsT=w2r_sb[:, hc * 128:(hc + 1) * 128],
                        rhs=hT[:, hc, :],
                        start=(hc == 0), stop=(hc == HC - 1))
                o_sb = outpool.tile([128, nn], OUT_DT, tag="o")
                nc.scalar.activation(o_sb[:], o_ps[:],
                                     mybir.ActivationFunctionType.Identity,
                                     bias=b2T_sb[:, 0:1], scale=1.0)
                nc.scalar.dma_start(y[:, n0:n0 + nn], o_sb[:])
                t0 += nts

    nc.compile()
    nc.m = get_hw_module(nc.m)
    return nc


# ---------------- host-side sharding / packing ----------------

def _pack_bins(deg):
    """Snake-assign degree-sorted nodes into B_GLOBAL bins of <=32 nodes
    and (statistically) <=1024 edges.  Returns bin id + position-in-bin
    per node."""
    order = np.argsort(-deg, kind="stable")
    B = B_GLOBAL
    bsum = np.zeros(B, np.int64)
    bn = np.zeros(B, np.int64)
    bin_of = np.empty(N_NODES, np.int64)
    pos_of = np.empty(N_NODES, np.int64)
    n = len(order)
    for r in range((n + B - 1) // B):
        take = order[r * B:(r + 1) * B]
        ob = np.argsort(bsum, kind="stable")[:len(take)]
        bin_of[take] = ob
        pos_of[take] = bn[ob]
        bn[ob] += 1
        bsum[ob] += deg[take]
    # safety: if any bin exceeds the 1024-edge cap, move its smallest-
    # degree nodes to the emptiest bins with node room
    while True:
        over = np.flatnonzero(bsum > CPB * 128)
        if not len(over):
            break
        b = over[0]
        nodes = np.flatnonzero(bin_of == b)
        v = nodes[np.argmin(deg[nodes])]
        cand = np.flatnonzero(bn < TN)
        tgt = cand[np.argmin(bsum[cand])]
        if bsum[tgt] + deg[v] > CPB * 128:
            raise RuntimeError("bin packing failed")
        # re-compact positions in source bin
        pos_of[nodes[pos_of[nodes] > pos_of[v]]] -= 1
        bin_of[v] = tgt
        pos_of[v] = bn[tgt]
        bn[tgt] += 1
        bn[b] -= 1
        bsum[tgt] += deg[v]
        bsum[b] -= deg[v]
    return bin_of, pos_of


def prepare_inputs(x, edge_attr, u, W1, b1, W2, b2, edge_index):
    x = np.asarray(x, dtype=np.float32)
    edge_attr = np.asarray(edge_attr, dtype=np.float32)
    u = np.asarray(u, dtype=np.float32)
    W1 = np.asarray(W1, dtype=np.float32)
    b1 = np.asarray(b1, dtype=np.float32)
    W2 = np.asarray(W2, dtype=np.float32)
    b2 = np.asarray(b2, dtype=np.float32)
    recv = np.asarray(edge_index)[1].astype(np.int64)

    edge_np = _np(EDGE_DT)
    idx_np = _np(IDX_DT)
    mlp_np = _np(MLP_DT)

    deg = np.bincount(recv, minlength=N_NODES)
    bin_of, pos_of = _pack_bins(deg)

    # sort edges by (bin, pos-in-bin) of their receiver -> per-node runs
    node_key = bin_of * TN + pos_of            # globally unique per node
    ekey = node_key[recv]
    order = np.argsort(ekey, kind="stable")
    ekey_s = ekey[order]
    ea_s = edge_attr[order]

    # error-feedback fp8 quantization per receiver run: the device's
    # fp32 sum of q equals the true fp32 sum minus the final carry
    run_start = np.r_[True, ekey_s[1:] != ekey_s[:-1]]
    starts = np.flatnonzero(run_start)
    run_id = np.cumsum(run_start) - 1
    pos_in_run = np.arange(len(ekey_s)) - starts[run_id]
    q_s = np.empty((len(ekey_s), F), edge_np)
    carry = np.zeros((len(starts), F), np.float32)
    for p in range(int(pos_in_run.max()) + 1):
        sel = np.flatnonzero(pos_in_run == p)
        if not len(sel):
            break
        r = run_id[sel]
        v = ea_s[sel] + carry[r]
        qv = v.astype(edge_np)
        carry[r] = v - qv.astype(np.float32)
        q_s[sel] = qv

    # slot within bin: rank of edge inside its bin
    ebin = ekey_s // TN
    bin_start = np.searchsorted(ebin, np.arange(B_GLOBAL))
    rank = np.arange(len(ebin)) - bin_start[ebin]
    # chunk within core: bin (local) * CPB + rank // 128
    core_of = ebin // NT
    q_local = (ebin % NT) * CPB + (rank >> 7)
    p_slot = rank & 127
    flat = p_slot * QT + q_local               # within-core flat slot

    # shared (replicated) tensors
    b1_eff = b1 + (u[0] @ W1[256:256 + D_U])
    w1a = np.ascontiguousarray(W1[0:128]).astype(mlp_np)
    w1b = np.ascontiguousarray(W1[128:256]).astype(mlp_np)
    w2r = np.ascontiguousarray(
        W2.reshape(HC, 128, F).transpose(1, 0, 2).reshape(128, H)
    ).astype(mlp_np)
    b1T = np.ascontiguousarray(b1_eff.reshape(HC, 128).T).astype(np.float32)
    b2T = b2.reshape(128, 1).astype(np.float32)
    iota_arr = np.tile(np.arange(TN, dtype=np.float32),
                       (128, 1)).astype(idx_np)

    in_maps = []
    node_slot = np.empty(N_NODES, np.int64)  # per-core slot of each node
    for c in range(N_CORES):
        esel = core_of == c
        ebuf = np.zeros((128 * QT, F), edge_np)
        ebuf[flat[esel]] = q_s[esel]
        rel = np.full(128 * QT, -1.0, np.float32)
        rel[flat[esel]] = (ekey_s[esel] % TN).astype(np.float32)

        nsel = np.flatnonzero((bin_of >= c * NT) & (bin_of < (c + 1) * NT))
        slots = (bin_of[nsel] - c * NT) * TN + pos_of[nsel]
        node_slot[nsel] = slots
        xT_arr = np.zeros((128, NODES_PAD), mlp_np)
        xT_arr[:, slots] = x[nsel].T.astype(mlp_np)

        in_maps.append({
            "edges": ebuf.reshape(128, QT, F),
            "relT": rel.reshape(128, QT).astype(idx_np),
            "iota": iota_arr, "xT": xT_arr,
            "w1a": w1a, "w1b": w1b, "w2r": w2r, "b1T": b1T, "b2T": b2T,
        })
    core_of_node = bin_of // NT
    return in_maps, core_of_node, node_slot


_prog_cache = {}


def _get_program():
    key = (EDGE_DT, MLP_DT, OUT_DT)
    if key not in _prog_cache:
        _prog_cache[key] = build_program()
    return _prog_cache[key]


def run(inputs, trace=False, tmpdir=None):
    in_maps, core_of_node, node_slot = prepare_inputs(**inputs)
    nc = _get_program()
    res = bass_utils.run_bass_kernel_spmd(
        nc, in_maps, core_ids=list(range(N_CORES)), trace=trace,
        tmpdir=tmpdir)
    out = np.empty((N_NODES, F), np.float32)
    for c in range(N_CORES):
        yc = np.asarray(res.results[c]["y"], dtype=np.float32)  # [128, PAD]
        nsel = np.flatnonzero(core_of_node == c)
        out[nsel] = yc[:, node_slot[nsel]].T
    return out, res


def kernel(**inputs) -> np.ndarray:
    out, _ = run(inputs, trace=False)
    return out


# revision 13
# speedup vs baseline: 1.0692x; 1.0692x over previous
"""GNN NodeBlock kernel for Trainium2, 8 NeuronCores (SPMD, no collectives).

Reference computation (N=50000 nodes, E=1600000 edges, F=128 features):
    recv_agg = segment_sum(edge_attr, edge_index[1], N)        # [N, 128]
    collected = concat([recv_agg, x, broadcast(u)], -1)        # [N, 272]
    out = relu(collected @ W1 + b1) @ W2 + b2                  # [N, 128]

Host-side sharding: nodes are re-balanced into 1584 bins of <=32 nodes
with <=1024 edges each (snake assignment of degree-sorted nodes), so
every bin needs exactly 8 edge-chunks of 128 — a fully static, uniform
program across all 8 cores (198 bins/core, 1584 chunks/core vs ~1758
for contiguous sharding).  Edges are packed fp8(e4m3) with
error-feedback quantization per receiver run: the device's fp32 PSUM
sum of the quantized stream telescopes to the true sum minus one final
sub-ulp carry (rel err ~6e-3 vs 2.9e-2 for plain fp8 rounding).  The
u-term of layer 1 is folded into b1 on the host.

Device program per core, one supertile (16 bins = 512 nodes) at a time:
  scatter: one DVE is_equal builds all 128 one-hot blocks
    onehot[e, q, n] = (rel[e, q] == n) in fp8; the PE accumulates
    aggT[f, 32-node window] += chunk^T @ onehot into a [128, 512] PSUM
    bank (8 chunks per bin, start/stop per bin).  fp8 weights get FWL
    (4 elem/cycle LDWEIGHTS), so the 32-wide matmuls run at the ~60
    cycle dispatch floor.
  L1: h_ps[h, n] = W1a^T @ aggT + W1b^T @ xT per 128-hidden chunk;
    ReLU+bias evacuation alternates scalar/vector engines.
  L2: o_ps[f, n] = sum_hc W2r_hc^T @ hT_hc (weights stationary,
    feature-major output); bias b2 added during PSUM evacuation; host
    transposes the output back to node-major.
Everything streams behind the PE: the edge DMA (2 MB fp8/supertile) and
all DVE/ACT work fit well under the ~8.4 us/supertile of matmul.
"""

import numpy as np

from concourse import bacc, mybir, tile
from concourse import bass_utils
from concourse.bass_interp import get_hw_module

# ---------------- problem constants (hardcoded per spec) ----------------
N_NODES = 50000
N_EDGES = 1600000
F = 128           # edge/node feature dim
H = 1024          # hidden dim
HC = H // 128     # 8 hidden chunks
D_U = 16
N_CORES = 8
TN = 32                                # nodes per bin (one-hot window)
NT = 198                               # bins per core
B_GLOBAL = N_CORES * NT                # 1584 bins
CPB = 8                                # chunks per bin (bin cap = 1024 edges)
QT = NT * CPB                          # 1584 chunks per core
NODES_PAD = NT * TN                    # 6336 node slots per core
SUP = 16                               # bins per supertile (512 nodes)
# graded startup: the NEFF's DMA path has ~10us of fixed init latency;
# small first supertiles let the PE start on a 0.25 MB slab instead of
# stalling on a 2 MB one
SUPERS = [2, 4, 8] + [SUP] * 11 + [NT - 14 - SUP * 11]  # [2,4,8,16*11,8]
assert sum(SUPERS) == NT
HEAD_BINS = 2 + 4 + 8 + 16             # head tiles cover supertiles 0-3
XH = HEAD_BINS * TN                    # 960 node cols in the x head tile
QH = HEAD_BINS * CPB                   # 240 chunks in the rel head tile

EDGE_DT = mybir.dt.float8e4            # ml_dtypes.float8_e4m3
IDX_DT = mybir.dt.bfloat16             # rel codes / iota (ints 0..31 exact)
MLP_DT = mybir.dt.bfloat16
OUT_DT = mybir.dt.bfloat16             # on-device output store dtype

_np = mybir.dt.np  # mybir dtype -> numpy dtype


# ---------------- device program ----------------

def build_program():
    f32 = mybir.dt.float32

    nc = bacc.Bacc("TRN2", target_bir_lowering=False, debug=False,
                   num_devices=N_CORES)

    edges = nc.dram_tensor("edges", [128, QT, F], EDGE_DT,
                           kind="ExternalInput").ap()
    relT = nc.dram_tensor("relT", [128, QT], IDX_DT,
                          kind="ExternalInput").ap()
    iota = nc.dram_tensor("iota", [128, TN], IDX_DT,
                          kind="ExternalInput").ap()
    xT = nc.dram_tensor("xT", [128, NODES_PAD], MLP_DT,
                        kind="ExternalInput").ap()
    w1a = nc.dram_tensor("w1a", [128, H], MLP_DT, kind="ExternalInput").ap()
    w1b = nc.dram_tensor("w1b", [128, H], MLP_DT, kind="ExternalInput").ap()
    w2r = nc.dram_tensor("w2r", [128, H], MLP_DT, kind="ExternalInput").ap()
    b1T = nc.dram_tensor("b1T", [128, HC], f32, kind="ExternalInput").ap()
    b2T = nc.dram_tensor("b2T", [128, 1], f32, kind="ExternalInput").ap()
    y = nc.dram_tensor("y", [128, NODES_PAD], OUT_DT,
                       kind="ExternalOutput").ap()

    with tile.TileContext(nc) as tc:
        with (
            tc.tile_pool(name="const", bufs=1) as cpool,
            tc.tile_pool(name="edge", bufs=4) as epool,
            tc.tile_pool(name="oh", bufs=3) as ohpool,
            tc.tile_pool(name="agg", bufs=2) as aggpool,
            tc.tile_pool(name="h", bufs=2) as hpool,
            tc.tile_pool(name="out", bufs=2) as outpool,
            tc.tile_pool(name="ps_agg", bufs=2, space="PSUM") as ps_agg,
            tc.tile_pool(name="ps_h", bufs=4, space="PSUM") as ps_h,
            tc.tile_pool(name="ps_out", bufs=2, space="PSUM") as ps_out,
        ):
            # Startup-critical constants ride the sync ring in exact
            # consumption order, interleaved with the first edge slab;
            # late-needed tensors (rel/x tails, W2) go on the scalar
            # ring.  Head/tail pairs are SEPARATE tiles (a single tile
            # written by two DMAs makes every reader wait for both).
            iota_sb = cpool.tile([128, TN], IDX_DT, tag="iota")
            nc.sync.dma_start(iota_sb[:], iota[:])
            relTh_sb = cpool.tile([128, QH], IDX_DT, tag="relTh")
            nc.sync.dma_start(relTh_sb[:], relT[:, :QH])

            iota_bc1 = iota_sb[:].rearrange("p (u n) -> p u n", u=1)

            # PE warm-up: engines start ~3us into the NEFF but the first
            # DMA bytes only land ~10us in.  Junk matmuls on a memset
            # scratch (no DMA dependency) keep HAM at 8/8 through that
            # window so the first real supertiles don't run at half
            # clock.
            scratch = cpool.tile([128, 512], MLP_DT, tag="scratch")
            nc.vector.memset(scratch[:], 0.0)
            warm_ps = ps_agg.tile([128, 512], f32, tag="agg")
            for w in range(26):
                nc.tensor.matmul(warm_ps[:],
                                 lhsT=scratch[:, :128], rhs=scratch[:],
                                 start=True, stop=True)

            # first edge slab, then the L1 weights + x/bias heads on the
            # same (favored) sync ring; W2 + tails on the scalar ring
            nsup = len(SUPERS)
            sup_q0 = [sum(SUPERS[:i]) * CPB for i in range(nsup)]
            sup_n0 = [sum(SUPERS[:i]) * TN for i in range(nsup)]
            e_sups = {}

            def load_slab(s, ring):
                nq_ = SUPERS[s] * CPB
                q0_ = sup_q0[s]
                e_ = epool.tile([128, nq_, F], EDGE_DT, tag="e")
                ring.dma_start(e_[:], edges[:, q0_:q0_ + nq_])
                e_sups[s] = e_

            load_slab(0, nc.sync)
            w1a_sb = cpool.tile([128, H], MLP_DT, tag="w1a")
            nc.sync.dma_start(w1a_sb[:], w1a[:])
            w1b_sb = cpool.tile([128, H], MLP_DT, tag="w1b")
            nc.sync.dma_start(w1b_sb[:], w1b[:])
            xTh_sb = cpool.tile([128, XH], MLP_DT, tag="xTh")
            nc.sync.dma_start(xTh_sb[:], xT[:, :XH])
            b1T_sb = cpool.tile([128, HC], f32, tag="b1T")
            nc.sync.dma_start(b1T_sb[:], b1T[:])

            w2r_sb = cpool.tile([128, H], MLP_DT, tag="w2r")
            nc.scalar.dma_start(w2r_sb[:], w2r[:])
            b2T_sb = cpool.tile([128, 1], f32, tag="b2T")
            nc.scalar.dma_start(b2T_sb[:], b2T[:])
            relTt_sb = cpool.tile([128, QT - QH], IDX_DT, tag="relTt")
            nc.scalar.dma_start(relTt_sb[:], relT[:, QH:])
            xTt_sb = cpool.tile([128, NODES_PAD - XH], MLP_DT, tag="xTt")
            nc.scalar.dma_start(xTt_sb[:], xT[:, XH:])

            def x_slice(n0_, nn_):
                if n0_ < XH:
                    return xTh_sb[:, n0_:n0_ + nn_]
                return xTt_sb[:, n0_ - XH:n0_ - XH + nn_]

            def make_onehot(s):
                nq_ = SUPERS[s] * CPB
                q0_ = sup_q0[s]
                oh_ = ohpool.tile([128, nq_, TN], EDGE_DT, tag="oh")
                if q0_ < QH:
                    rel = relTh_sb[:, q0_:q0_ + nq_]
                else:
                    rel = relTt_sb[:, q0_ - QH:q0_ - QH + nq_]
                rel_bc = rel.rearrange(
                    "p (c u) -> p c u", u=1).broadcast_to([128, nq_, TN])
                nc.vector.tensor_tensor(
                    out=oh_[:], in0=iota_bc1.broadcast_to([128, nq_, TN]),
                    in1=rel_bc, op=mybir.AluOpType.is_equal)
                return oh_

            def scatter(s):
                nts_ = SUPERS[s]
                nn_ = nts_ * TN
                e_ = e_sups.pop(s)
                oh_ = ohs.pop(s)
                agg_ps_ = ps_agg.tile([128, nn_], f32, tag="agg")
                for st in range(nts_):
                    for c in range(CPB):
                        q = st * CPB + c
                        nc.tensor.matmul(
                            agg_ps_[:, st * TN:(st + 1) * TN],
                            lhsT=e_[:, q, :],
                            rhs=oh_[:, q, :],
                            start=(c == 0),
                            stop=(c == CPB - 1),
                        )
                aggT_ = aggpool.tile([128, nn_], MLP_DT, tag="aggT")
                nc.scalar.copy(aggT_[:], agg_ps_[:])
                return aggT_

            def mlp(s, aggT_):
                nn_ = SUPERS[s] * TN
                n0_ = sup_n0[s]
                xs = x_slice(n0_, nn_)
                hT = hpool.tile([128, HC, nn_], MLP_DT, tag="hT")
                for hc in range(HC):
                    h_ps = ps_h.tile([128, nn_], f32, tag="h")
                    nc.tensor.matmul(h_ps[:],
                                     lhsT=w1b_sb[:, hc * 128:(hc + 1) * 128],
                                     rhs=xs,
                                     start=True, stop=False)
                    nc.tensor.matmul(h_ps[:],
                                     lhsT=w1a_sb[:, hc * 128:(hc + 1) * 128],
                                     rhs=aggT_[:],
                                     start=False, stop=True)
                    if hc % 2 == 0:
                        nc.scalar.activation(
                            hT[:, hc, :], h_ps[:],
                            mybir.ActivationFunctionType.Relu,
                            bias=b1T_sb[:, hc:hc + 1], scale=1.0)
                    else:
                        nc.vector.tensor_scalar(
                            out=hT[:, hc, :], in0=h_ps[:],
                            scalar1=b1T_sb[:, hc:hc + 1], scalar2=0.0,
                            op0=mybir.AluOpType.add,
                            op1=mybir.AluOpType.max)
                o_ps = ps_out.tile([128, nn_], f32, tag="ops")
                for hc in range(HC):
                    nc.tensor.matmul(
                        o_ps[:],
                        lhsT=w2r_sb[:, hc * 128:(hc + 1) * 128],
                        rhs=hT[:, hc, :],
                        start=(hc == 0), stop=(hc == HC - 1))
                o_sb = outpool.tile([128, nn_], OUT_DT, tag="o")
                nc.scalar.activation(o_sb[:], o_ps[:],
                                     mybir.ActivationFunctionType.Identity,
                                     bias=b2T_sb[:, 0:1], scale=1.0)
                nc.scalar.dma_start(y[:, n0_:n0_ + nn_], o_sb[:])

            # Software-pipelined main loop: the MLP of supertile s-1 runs
            # AFTER the scatter of supertile s, so the aggT evacuation
            # (ACT engine) and the next one-hot (DVE) always have a full
            # scatter's worth of PE time to hide behind.
            ohs = {0: make_onehot(0), 1: make_onehot(1)}
            aggs = {}
            for s in range(nsup):
                if s + 1 < nsup:
                    load_slab(s + 1, nc.sync)
                aggs[s] = scatter(s)
                if s + 2 < nsup:
                    ohs[s + 2] = make_onehot(s + 2)
                if s - 1 in aggs:
                    mlp(s - 1, aggs.pop(s - 1))
            mlp(nsup - 1, aggs.pop(nsup - 1))

    nc.compile()
    nc.m = get_hw_module(nc.m)
    return nc


# ---------------- host-side sharding / packing ----------------

def _pack_bins(deg):
    """Snake-assign degree-sorted nodes into B_GLOBAL bins of <=32 nodes
    and (statistically) <=1024 edges.  Returns bin id + position-in-bin
    per node."""
    order = np.argsort(-deg, kind="stable")
    B = B_GLOBAL
    bsum = np.zeros(B, np.int64)
    bn = np.zeros(B, np.int64)
    bin_of = np.empty(N_NODES, np.int64)
    pos_of = np.empty(N_NODES, np.int64)
    n = len(order)
    for r in range((n + B - 1) // B):
        take = order[r * B:(r + 1) * B]
        ob = np.argsort(bsum, kind="stable")[:len(take)]
        bin_of[take] = ob
        pos_of[take] = bn[ob]
        bn[ob] += 1
        bsum[ob] += deg[take]
    # safety: if any bin exceeds the 1024-edge cap, move its smallest-
    # degree nodes to the emptiest bins with node room
    while True:
        over = np.flatnonzero(bsum > CPB * 128)
        if not len(over):
            break
        b = over[0]
        nodes = np.flatnonzero(bin_of == b)
        v = nodes[np.argmin(deg[nodes])]
        cand = np.flatnonzero(bn < TN)
        tgt = cand[np.argmin(bsum[cand])]
        if bsum[tgt] + deg[v] > CPB * 128:
            raise RuntimeError("bin packing failed")
        # re-compact positions in source bin
        pos_of[nodes[pos_of[nodes] > pos_of[v]]] -= 1
        bin_of[v] = tgt
        pos_of[v] = bn[tgt]
        bn[tgt] += 1
        bn[b] -= 1
        bsum[tgt] += deg[v]
        bsum[b] -= deg[v]
    return bin_of, pos_of


def prepare_inputs(x, edge_attr, u, W1, b1, W2, b2, edge_index):
    x = np.asarray(x, dtype=np.float32)
    edge_attr = np.asarray(edge_attr, dtype=np.float32)
    u = np.asarray(u, dtype=np.float32)
    W1 = np.asarray(W1, dtype=np.float32)
    b1 = np.asarray(b1, dtype=np.float32)
    W2 = np.asarray(W2, dtype=np.float32)
    b2 = np.asarray(b2, dtype=np.float32)
    recv = np.asarray(edge_index)[1].astype(np.int64)

    edge_np = _np(EDGE_DT)
    idx_np = _np(IDX_DT)
    mlp_np = _np(MLP_DT)

    deg = np.bincount(recv, minlength=N_NODES)
    bin_of, pos_of = _pack_bins(deg)

    # sort edges by (bin, pos-in-bin) of their receiver -> per-node runs
    node_key = bin_of * TN + pos_of            # globally unique per node
    ekey = node_key[recv]
    order = np.argsort(ekey, kind="stable")
    ekey_s = ekey[order]
    ea_s = edge_attr[order]

    # error-feedback fp8 quantization per receiver run: the device's
    # fp32 sum of q equals the true fp32 sum minus the final carry
    run_start = np.r_[True, ekey_s[1:] != ekey_s[:-1]]
    starts = np.flatnonzero(run_start)
    run_id = np.cumsum(run_start) - 1
    pos_in_run = np.arange(len(ekey_s)) - starts[run_id]
    q_s = np.empty((len(ekey_s), F), edge_np)
    carry = np.zeros((len(starts), F), np.float32)
    for p in range(int(pos_in_run.max()) + 1):
        sel = np.flatnonzero(pos_in_run == p)
        if not len(sel):
            break
        r = run_id[sel]
        v = ea_s[sel] + carry[r]
        qv = v.astype(edge_np)
        carry[r] = v - qv.astype(np.float32)
        q_s[sel] = qv

    # slot within bin: rank of edge inside its bin
    ebin = ekey_s // TN
    bin_start = np.searchsorted(ebin, np.arange(B_GLOBAL))
    rank = np.arange(len(ebin)) - bin_start[ebin]
    # chunk within core: bin (local) * CPB + rank // 128
    core_of = ebin // NT
    q_local = (ebin % NT) * CPB + (rank >> 7)
    p_slot = rank & 127
    flat = p_slot * QT + q_local               # within-core flat slot

    # shared (replicated) tensors
    b1_eff = b1 + (u[0] @ W1[256:256 + D_U])
    w1a = np.ascontiguousarray(W1[0:128]).astype(mlp_np)
    w1b = np.ascontiguousarray(W1[128:256]).astype(mlp_np)
    w2r = np.ascontiguousarray(
        W2.reshape(HC, 128, F).transpose(1, 0, 2).reshape(128, H)
    ).astype(mlp_np)
    b1T = np.ascontiguousarray(b1_eff.reshape(HC, 128).T).astype(np.float32)
    b2T = b2.reshape(128, 1).astype(np.float32)
    iota_arr = np.tile(np.arange(TN, dtype=np.float32),
                       (128, 1)).astype(idx_np)

    in_maps = []
    node_slot = np.empty(N_NODES, np.int64)  # per-core slot of each node
    for c in range(N_CORES):
        esel = core_of == c
        ebuf = np.zeros((128 * QT, F), edge_np)
        ebuf[flat[esel]] = q_s[esel]
        rel = np.full(128 * QT, -1.0, np.float32)
        rel[flat[esel]] = (ekey_s[esel] % TN).astype(np.float32)

        nsel = np.flatnonzero((bin_of >= c * NT) & (bin_of < (c + 1) * NT))
        slots = (bin_of[nsel] - c * NT) * TN + pos_of[nsel]
        node_slot[nsel] = slots
        xT_arr = np.zeros((128, NODES_PAD), mlp_np)
        xT_arr[:, slots] = x[nsel].T.astype(mlp_np)

        in_maps.append({
            "edges": ebuf.reshape(128, QT, F),
            "relT": rel.reshape(128, QT).astype(idx_np),
            "iota": iota_arr, "xT": xT_arr,
            "w1a": w1a, "w1b": w1b, "w2r": w2r, "b1T": b1T, "b2T": b2T,
        })
    core_of_node = bin_of // NT
    return in_maps, core_of_node, node_slot


_prog_cache = {}


def _get_program():
    key = (EDGE_DT, MLP_DT, OUT_DT)
    if key not in _prog_cache:
        _prog_cache[key] = build_program()
    return _prog_cache[key]


def run(inputs, trace=False, tmpdir=None):
    in_maps, core_of_node, node_slot = prepare_inputs(**inputs)
    nc = _get_program()
    res = bass_utils.run_bass_kernel_spmd(
        nc, in_maps, core_ids=list(range(N_CORES)), trace=trace,
        tmpdir=tmpdir)
    out = np.empty((N_NODES, F), np.float32)
    for c in range(N_CORES):
        yc = np.asarray(res.results[c]["y"], dtype=np.float32)  # [128, PAD]
        nsel = np.flatnonzero(core_of_node == c)
        out[nsel] = yc[:, node_slot[nsel]].T
    return out, res


def kernel(**inputs) -> np.ndarray:
    out, _ = run(inputs, trace=False)
    return out
